# revision 25
# baseline (speedup 1.0000x reference)
"""Trainium2 Bass kernel for int8-dequant Linear: out = x @ (W_q * scaler)^T.

Full shapes: x [4, 2048, 4096] f32, weight_q [4096, 4096] int8,
weight_scaler [4096] f32 -> out [4, 2048, 4096] f32.

Sharding: data-parallel over tokens (8192 tokens -> 1024 per core);
weight_q/scaler replicated. Each core computes out.T for its token
shard with out-channels on PSUM partitions; the per-channel scaler is
applied as a per-partition scalar multiply on PSUM eviction.

Matmul dtype (MODE):
  "fp16"  - x and W both fp16. W int8-valued: exact (11-bit
            significand); x rounded to fp16: rel err ~5e-4, better
            than f32r's ~1.4e-3. 1 cyc/row on the PE, and the 2-byte
            LDWEIGHTS fully hides behind the 512-row matmul stream
            (f32r's 4-byte stationary load is ~187ns and stretches
            the steady-state period from 213ns to 227ns).
  "wbf16" - W bf16 stationary, x f32r moving (rel err ~1.4e-3).
  "f32r"  - both f32r (original baseline numerics).

Schedule notes (measured on hw via perfetto):
  - DMA descriptor issue (DIRECT2D) is ~0.65us serial per sequencer,
    the 16 DMA queues arm serially ~8.5-10.5us into the kernel, and
    each queue round-robins 8-line bursts between the Sync-issued and
    Activation-issued rings.  The k=0-critical programs are spread
    over both rings; later x tiles ride Sync and weights Activation.
  - PE warm-up dummy matmuls burn the 0.65->1.2->2.4GHz DVFS ramp
    while the first DMAs land (a >0.7us PE gap resets the ramp and
    costs ~2.4us of mid-clock matmuls).
  - The four h=0 weight casts are emitted as half-quarter slices,
    first halves for every mo first, so k=0 unlocks after 4x0.35us.
  - Zero-stationary pad matmuls (accumulate +0 into the already-open
    PSUM group) absorb cast arrival jitter at the k=0 seams.
  - Output is written as fp16 (|out| <= ~300, fp16 rounding adds
    ~1e-3 rel err): halves output DMA and doubles DVE evict rate.
  - Evictions alternate DVE (n=0) / Activation (n=1) and their DMA
    issues alternate Sync/Activation, so the final mo-tile's two
    evictions and issues run pairwise in parallel after the last
    matmul.
  - Steady state: 2048 matmuls x 215.7ns, zero PE idle gaps; fp16
    LDWEIGHTS (~97ns) hides fully behind the 512-row stream.
"""
import sys

sys.path.insert(0, "/opt/trn_rl_repo")

import numpy as np

import concourse.bacc as bacc
import concourse.mybir as mybir
import concourse.tile as tile
from concourse.bass_utils import run_bass_kernel_spmd

N_CORES = 8
P = 128
IN_F = 4096
OUT_F = 4096
TOKENS = 4 * 2048
T_SHARD = TOKENS // N_CORES          # 1024 tokens per core
KT = IN_F // P                       # 32 k-tiles
MT = OUT_F // P                      # 32 m-tiles (out-channel tiles)
N_FREE = 512                         # moving free dim per matmul (1 PSUM bank)
NT = T_SHARD // N_FREE               # 2 n-tiles

MODE = "fp16"                        # "fp16" | "wbf16" | "bf16" | "f32r"

WARM_NS = 3                          # small (128-row) warm-up matmuls
WARM_NB = 10                         # big (512-row) warm-up matmuls
WARM_NT = 0                          # trailing small warm-up matmuls

_cache = {}


def _build(mode):
    f32 = mybir.dt.float32
    if mode == "fp16":
        x_dt = w_dt = mybir.dt.float16
    elif mode == "bf16":
        x_dt = w_dt = mybir.dt.bfloat16
    elif mode == "wbf16":
        x_dt, w_dt = mybir.dt.float32r, mybir.dt.bfloat16
    else:
        x_dt = w_dt = mybir.dt.float32r

    nc = bacc.Bacc(None, target_bir_lowering=False, debug=False)

    d_x = nc.declare_dram_parameter("xq0", [IN_F, T_SHARD], x_dt, isOutput=False)
    # Weights travel as int8 (4x less DMA) and are upconverted to the
    # stationary dtype on-chip by the otherwise-idle DVE.
    d_w = nc.declare_dram_parameter("wq4", [MT, P, KT, P], mybir.dt.int8, isOutput=False)
    d_s = nc.declare_dram_parameter("scal", [P, MT], f32, isOutput=False)
    o_dt = mybir.dt.float16 if mode == "fp16" else f32
    d_o = nc.declare_dram_parameter("outT", [MT, P, T_SHARD], o_dt, isOutput=True)

    WH = 8                       # k-tiles per weight quarter-tile
    NH = KT // WH                # weight quarter-tiles per mo
    PRO = 4                      # mo-tiles interleaved during the x load

    with tile.TileContext(nc) as tc:
        with (
            tc.tile_pool(name="xh", bufs=NT) as xhp,
            tc.tile_pool(name="xp", bufs=KT - 1) as xp,
            tc.tile_pool(name="wp", bufs=12) as wp,
            tc.tile_pool(name="ws", bufs=8) as ws,
            tc.tile_pool(name="op", bufs=6) as op,
            tc.tile_pool(name="cp", bufs=5) as cp,
            tc.tile_pool(name="ps", bufs=8, space="PSUM") as ps,
        ):
            # PE warm-up / pad sources (memset on the idle DVE; the
            # small zero_w first so the first dummy matmul can start
            # as early as possible).
            zero_w = cp.tile([P, P], w_dt, name="zero_w")
            nc.vector.memset(zero_w[:], 0.0)
            warm = cp.tile([P, N_FREE], w_dt, name="warm")
            nc.vector.memset(warm[:], 1.0)

            def w_half(mo, h, issue=None, cast=True):
                s = ws.tile([P, WH, P], mybir.dt.int8, tag="w8", name=f"w8_{mo}_{h}")
                (issue or nc.scalar).dma_start(
                    s[:], d_w.ap()[mo, :, h * WH:(h + 1) * WH, :]
                )
                t = wp.tile([P, WH, P], w_dt, tag="wh", name=f"wh_{mo}_{h}")
                if cast:
                    nc.vector.tensor_copy(t[:], s[:])
                    return t
                return s, t

            xht = {}                 # n -> [P, N_FREE] half tile for k=0
            xt = [None] * KT         # k -> [P, T_SHARD] tile, k >= 1

            def x_half(n):
                t = xhp.tile([P, N_FREE], x_dt, tag="xht", name=f"xh_0_{n}")
                nc.sync.dma_start(
                    t[:], d_x.ap()[0:P, n * N_FREE:(n + 1) * N_FREE]
                )
                xht[n] = t

            def x_tile(k):
                t = xp.tile([P, T_SHARD], x_dt, tag="xt", name=f"xt_{k}")
                nc.sync.dma_start(t[:], d_x.ap()[k * P:(k + 1) * P, :])
                xt[k] = t

            def x_slice(k, n):
                if k == 0:
                    return xht[n][:]
                return xt[k][:, n * N_FREE:(n + 1) * N_FREE]

            def evict(mo, n, psum, scal):
                # n=0 on the DVE, n=1 on the Activation engine
                # (out = Copy(psum * scale)): the final mo-tile's two
                # evictions run in parallel instead of serializing on
                # the DVE after the last matmul.
                osb = op.tile([P, N_FREE], o_dt, tag="osb", name=f"osb_{mo}_{n}")
                if n == 0:
                    nc.vector.tensor_scalar_mul(osb[:], psum[:], scal[:, mo:mo + 1])
                else:
                    nc.scalar.activation(
                        osb[:], psum[:],
                        mybir.ActivationFunctionType.Copy,
                        scale=scal[:, mo:mo + 1],
                    )
                # n=0's DMA issues from the (tail-idle) Sync sequencer
                # so the final two output issues go out in parallel.
                (nc.sync if n == 0 else nc.scalar).dma_start(
                    d_o.ap()[mo, :, n * N_FREE:(n + 1) * N_FREE], osb[:]
                )

            # --- head DMA issues.  The k=0 critical programs (four h=0
            # weight quarters + the two x halves) are spread over both
            # sequencer rings so the per-queue ring round-robin serves
            # them all within ~1us of queue arming.
            pairs = {}
            pairs[0] = w_half(0, 0, issue=nc.scalar, cast=False)
            x_half(0)                                   # sync
            pairs[1] = w_half(1, 0, issue=nc.sync, cast=False)
            pairs[2] = w_half(2, 0, issue=nc.scalar, cast=False)
            x_half(1)                                   # sync
            pairs[3] = w_half(3, 0, issue=nc.scalar, cast=False)
            # Cast the h=0 quarters in half-quarter slices, first halves
            # (k=0..3) for every mo before any second half: k=0 becomes
            # runnable after four 0.35us casts instead of four 0.7us.
            HW2 = WH // 2
            for mo in range(PRO):
                s, t = pairs[mo]
                nc.vector.tensor_copy(t[:, 0:HW2, :], s[:, 0:HW2, :])
            for mo in range(PRO):
                s, t = pairs[mo]
                nc.vector.tensor_copy(t[:, HW2:WH, :], s[:, HW2:WH, :])
            wh_pro = {(mo, 0): pairs[mo][1] for mo in range(PRO)}
            for k in range(1, WH):
                x_tile(k)
            scal = cp.tile([P, MT], f32, name="scal")
            nc.scalar.dma_start(scal[:], d_s.ap())

            # remaining k-groups: quarter weights just ahead of their x
            # tiles; the last group also prefetches mo=PRO's quarters so
            # phase 2 starts without waiting behind the x tail.
            wh_next = {}
            for h in range(1, NH):
                for mo in range(PRO):
                    wh_pro[(mo, h)] = w_half(mo, h)
                if h == NH - 1:
                    for j in range(NH):
                        wh_next[j] = w_half(PRO, j)
                for k in range(h * WH, (h + 1) * WH):
                    x_tile(k)

            # --- PE warm-up: dummy matmuls with no DMA deps keep the
            # clock ramping until the first weight cast lands (the
            # first real matmul can't start before ~11.5-12.5us: DMA
            # queue arming staggers to ~10.4us and the cast adds 0.7).
            warm_ps = ps.tile([P, N_FREE], f32, tag="psum", name="psum_warm")
            for i in range(WARM_NS):
                nc.tensor.matmul(
                    warm_ps[:, 0:P], zero_w[:], zero_w[:], start=True, stop=True
                )
            for i in range(WARM_NB):
                nc.tensor.matmul(
                    warm_ps[:], zero_w[:], warm[:], start=True, stop=True
                )
            for i in range(WARM_NT):
                nc.tensor.matmul(
                    warm_ps[:, 0:P], zero_w[:], zero_w[:], start=True, stop=True
                )
            # fp8 DoubleRow throughput probe: slice duration in the
            # trace tells whether K=256 double-pumped matmuls stream at
            # 0.5 or 1.0 cycles per output row on real TRN2 hw.
            f8 = mybir.dt.float8e4
            w8p = cp.tile([P, 2, P], f8, name="w8probe")
            nc.vector.memset(w8p[:], 0.0)
            x8p = cp.tile([P, 2, N_FREE], f8, name="x8probe")
            nc.vector.memset(x8p[:], 1.0)
            for i in range(6):
                nc.tensor.matmul(
                    warm_ps[:], w8p[:], x8p[:],
                    start=True, stop=True,
                    perf_mode=mybir.MatmulPerfMode.DoubleRow,
                )

            # --- phase 1 matmuls: first PRO mo-tiles, k-major so the PE
            # has work for every x k-tile as it lands.
            pro_ps = {
                (mo, n): ps.tile([P, N_FREE], f32, tag="psum", name=f"psum_{mo}_{n}")
                for mo in range(PRO)
                for n in range(NT)
            }
            # Request mo=PRO's banks now so the allocator binds them to
            # the earliest-released phase-1 banks.
            early_ps = [
                ps.tile([P, N_FREE], f32, tag="psum", name=f"psum_{PRO}_{n}")
                for n in range(NT)
            ]
            # k-major so the PE has work for every x k-tile as it
            # lands.  k=0 is paced by the four weight-cast arrivals
            # (~0.7-0.9us apart); +0 pads into the previous mo's open
            # group absorb that jitter without idling the PE (an idle
            # gap >~0.5us resets the DVFS ramp, costing ~2.4us).
            for k in range(KT):
                h, kh = divmod(k, WH)
                for mo in range(PRO):
                    if k == 0 and mo > 0:
                        for _ in range(1):
                            nc.tensor.matmul(
                                pro_ps[(mo - 1, 0)][:], zero_w[:], warm[:],
                                start=False, stop=False,
                            )
                    for n in range(NT):
                        nc.tensor.matmul(
                            pro_ps[(mo, n)][:],
                            wh_pro[(mo, h)][:, kh, :],
                            x_slice(k, n),
                            start=(k == 0),
                            stop=(k == KT - 1),
                        )
            for mo in range(PRO):
                for n in range(NT):
                    evict(mo, n, pro_ps[(mo, n)], scal)

            # --- phase 2: remaining mo-tiles, weight-reuse order.
            for mo in range(PRO, MT):
                if mo == PRO:
                    whs = [wh_next[h] for h in range(NH)]
                    psums = early_ps
                else:
                    whs = [w_half(mo, h) for h in range(NH)]
                    psums = [
                        ps.tile([P, N_FREE], f32, tag="psum", name=f"psum_{mo}_{n}")
                        for n in range(NT)
                    ]
                for k in range(KT):
                    h, kh = divmod(k, WH)
                    for n in range(NT):
                        nc.tensor.matmul(
                            psums[n][:],
                            whs[h][:, kh, :],
                            x_slice(k, n),
                            start=(k == 0),
                            stop=(k == KT - 1),
                        )
                for n in range(NT):
                    evict(mo, n, psums[n], scal)

    nc.compile()
    return nc


def _prep_inputs(x, weight_q, weight_scaler, mode):
    """Host-side shard + layout. Returns in_maps (list of dicts, one per core)."""
    xf = np.asarray(x, dtype=np.float32).reshape(TOKENS, IN_F)
    wq = np.asarray(weight_q)
    sc = np.asarray(weight_scaler, dtype=np.float32)

    # W tiles: w4[mo, p_in, ko, oc] = W[mo*128+oc, ko*128+p_in]
    # (matches the SBUF lhsT tile AP [P, KT, P] exactly), shipped as int8
    # and upconverted on-chip.
    w4 = np.ascontiguousarray(
        wq.reshape(MT, P, KT, P).transpose(0, 3, 2, 1)
    ).astype(np.int8)

    scal = np.ascontiguousarray(sc.reshape(MT, P).T)  # [P, MT]

    if mode == "fp16":
        x_cast = lambda a: a.astype(np.float16)
    elif mode == "bf16":
        import ml_dtypes

        x_cast = lambda a: a.astype(ml_dtypes.bfloat16)
    else:
        x_cast = lambda a: a

    in_maps = []
    for c in range(N_CORES):
        xs = xf[c * T_SHARD:(c + 1) * T_SHARD, :]      # [T_SHARD, IN_F]
        xsT = np.ascontiguousarray(xs.T)                # [IN_F, T_SHARD] f32
        in_maps.append({"wq4": w4, "scal": scal, "xq0": x_cast(xsT)})
    return in_maps


def _gather(results):
    """Per-core outT [MT, P, T_SHARD] -> full out [4, 2048, OUT_F] f32."""
    parts = []
    for c in range(N_CORES):
        ot = np.asarray(results[c]["outT"], dtype=np.float32)  # [MT, P, T_SHARD]
        parts.append(ot.reshape(OUT_F, T_SHARD).T)  # [T_SHARD, OUT_F]
    out = np.concatenate(parts, axis=0)           # [TOKENS, OUT_F]
    return np.ascontiguousarray(out.reshape(4, 2048, OUT_F), dtype=np.float32)


def _run(inputs, trace=False, mode=None):
    mode = mode or MODE
    if mode not in _cache:
        _cache[mode] = _build(mode)
    nc = _cache[mode]
    in_maps = _prep_inputs(inputs["x"], inputs["weight_q"], inputs["weight_scaler"], mode)
    res = run_bass_kernel_spmd(nc, in_maps, list(range(N_CORES)), trace=trace)
    return _gather(res.results), res


def kernel(**inputs):
    out, _ = _run(inputs, trace=False)
    return out
